# Initial kernel scaffold
#
"""Trainium2 Bass kernel for DensityMatrixMLP.

Computes, for each batch row b of x [B=131072, 256]:
    h   = relu(x @ W1 + b1)            # [128]
    v   = h @ W2 + b2                  # [136] = tril entries of L [16,16]
    rho = L @ L^T                      # [16,16]
    out = rho / trace(rho)

Strategy (pure data parallel over 8 NeuronCores, 16384 rows/core):
The per-batch Gram L@L^T is a fixed quadratic map of v. Every needed
product v_a*v_b is obtained from squares of *linear* channels
(polarization):  v_a*v_b = ((v_a+v_b)^2 - v_a^2 - v_b^2)/2.
So the whole kernel becomes constant-weight matmuls + one elementwise
square, which stream batch columns through the PE at full rate:

  x^T   (PE transpose)                               [256, NB]
  h^T  = relu(W1^T x^T + b1)                         [128, NB]
  w    = A^T h + d   (816 linear channels:           [816, NB]
         136 v-channels + 680 pairwise-sum channels; A = W2-derived)
  u    = (w + d)^2   (ACT/DVE elementwise square)    [816, NB]
  Tlow = C_low^T u   (120 strict-lower rho entries)  [120, NB]
  Tdia = C_diag^T u  (16 diag entries + trace)       [17, NB]
  out  = gather-transpose: per 128-batch chunk, matmul with 0/1
         gather matrices G0/G1 maps feature-major rows to batch-major
         [128b, 257] (256 rho entries + trace), then divide by trace.

All big matmuls run as float32r (full fp32 storage, fast PE mode).
"""

import sys

if "/opt/trn_rl_repo" not in sys.path:
    sys.path.insert(0, "/opt/trn_rl_repo")

from contextlib import ExitStack

import numpy as np

import concourse.bass as bass
import concourse.tile as tile
from concourse import bacc, mybir
from concourse.bass_utils import run_bass_kernel_spmd

# Problem shapes (hardcoded per spec).
BATCH = 131072
IN_DIM = 256
HID = 128
DIM = 16
TRIL = DIM * (DIM + 1) // 2  # 136
NCORES = 8
B_CORE = BATCH // NCORES  # 16384
NB = 512  # batch macro-tile (columns streamed per matmul)
NTILES = B_CORE // NB  # 32

# Channel space: 136 v-channels + 680 cross-sum channels.
NCROSS = sum(j + 1 for i in range(DIM) for j in range(i))  # 680
NCH = TRIL + NCROSS  # 816
NCH_PAD = 896  # 7 chunks of 128
NCHUNK = 7
NLOW = DIM * (DIM - 1) // 2  # 120
NOUT = DIM * DIM  # 256

F32 = mybir.dt.float32
F32R = mybir.dt.float32r


def _tidx(i, k):
    return i * (i + 1) // 2 + k


def _lowidx(i, j):
    return i * (i - 1) // 2 + j


def _cross_pairs():
    """Enumerate cross channels: (i, j, k) with i>j, k<=j."""
    out = []
    for i in range(DIM):
        for j in range(i):
            for k in range(j + 1):
                out.append((i, j, k))
    return out


def build_constants(W1, b1, W2, b2):
    """Host-side constant tensors, all laid out [128 partitions, free]."""
    W1 = np.asarray(W1, np.float32)
    b1 = np.asarray(b1, np.float32)
    W2 = np.asarray(W2, np.float32)
    b2 = np.asarray(b2, np.float32)
    pairs = _cross_pairs()

    # A [HID, NCH_PAD]: channel weights; d [NCH_PAD]: channel bias.
    A = np.zeros((HID, NCH_PAD), np.float32)
    d = np.zeros(NCH_PAD, np.float32)
    A[:, :TRIL] = W2
    d[:TRIL] = b2
    for m, (i, j, k) in enumerate(pairs):
        a, b = _tidx(i, k), _tidx(j, k)
        A[:, TRIL + m] = W2[:, a] + W2[:, b]
        d[TRIL + m] = b2[a] + b2[b]

    # C_low [NCH_PAD, NLOW]
    C_low = np.zeros((NCH_PAD, NLOW), np.float32)
    for m, (i, j, k) in enumerate(pairs):
        r = _lowidx(i, j)
        a, b = _tidx(i, k), _tidx(j, k)
        C_low[TRIL + m, r] += 0.5
        C_low[a, r] -= 0.5
        C_low[b, r] -= 0.5

    # C_diag [256 (2 chunks), 17]: cols 0..15 = rho_ii, col 16 = trace.
    C_diag = np.zeros((2 * 128, 17), np.float32)
    for i in range(DIM):
        for k in range(i + 1):
            C_diag[_tidx(i, k), i] = 1.0
    C_diag[:TRIL, 16] = 1.0

    # Gather matrices for the final feature-major -> batch-major matmul.
    G0 = np.zeros((128, NOUT + 1), np.float32)  # rows 0..119 used
    G1 = np.zeros((128, NOUT + 1), np.float32)  # rows 0..16 used
    for i in range(DIM):
        for j in range(DIM):
            p = i * DIM + j
            if i > j:
                G0[_lowidx(i, j), p] = 1.0
            elif i < j:
                G0[_lowidx(j, i), p] = 1.0
            else:
                G1[i, p] = 1.0
    G1[16, NOUT] = 1.0  # trace passthrough

    ident = np.eye(128, dtype=np.float32)

    # SBUF-friendly packing: [128, ...free], chunk index in free dim.
    w1c = np.zeros((128, 2, HID), np.float32)
    for c in range(2):
        w1c[:, c, :] = W1[c * 128 : (c + 1) * 128, :]
    A_p = A  # already [128, 896]
    dbias = np.zeros((128, NCHUNK), np.float32)
    for c in range(NCHUNK):
        dbias[:, c] = d[c * 128 : (c + 1) * 128]
    clow = np.zeros((128, NCHUNK, NLOW), np.float32)
    for c in range(NCHUNK):
        clow[:, c, :] = C_low[c * 128 : (c + 1) * 128, :]
    cdiag = np.zeros((128, 2, 17), np.float32)
    for c in range(2):
        cdiag[:, c, :] = C_diag[c * 128 : (c + 1) * 128, :]
    b1p = b1.reshape(128, 1).astype(np.float32)

    return {
        "w1c": w1c,
        "a_mat": np.ascontiguousarray(A_p),
        "dbias": dbias,
        "clow": clow,
        "cdiag": cdiag,
        "g0": G0,
        "g1": G1,
        "ident": ident,
        "b1p": b1p,
    }, bool(np.any(d != 0.0))


def emulate(x, consts):
    """Numpy emulation of the kernel math (for constant validation)."""
    w1c = consts["w1c"]
    W1 = np.concatenate([w1c[:, 0, :], w1c[:, 1, :]], axis=0)
    h = np.maximum(x @ W1 + consts["b1p"].ravel(), 0.0)
    A = consts["a_mat"]
    d = consts["dbias"].T.ravel()  # [7*128] chunk-major == channel index
    w = h @ A + d
    u = w * w
    C_low = consts["clow"].transpose(1, 0, 2).reshape(NCH_PAD, NLOW)
    C_diag = consts["cdiag"].transpose(1, 0, 2).reshape(256, 17)
    tlow = u @ C_low
    tdia = u[:, :256] @ C_diag
    o = tlow @ consts["g0"][:NLOW, :] + tdia @ consts["g1"][:17, :]
    return (o[:, :NOUT] / o[:, NOUT:]).reshape(-1, DIM, DIM)


def build_program(bias_d_nonzero, mm_dt=F32R):
    """Build the Bass/Tile program (value-independent)."""
    nc = bacc.Bacc("TRN2", target_bir_lowering=False, debug=False)

    x_d = nc.dram_tensor("x", [B_CORE, IN_DIM], F32, kind="ExternalInput").ap()
    out_d = nc.dram_tensor("out", [B_CORE, DIM, DIM], F32, kind="ExternalOutput").ap()
    cshapes = {
        "w1c": [128, 2, HID],
        "a_mat": [128, NCH_PAD],
        "dbias": [128, NCHUNK],
        "clow": [128, NCHUNK, NLOW],
        "cdiag": [128, 2, 17],
        "g0": [128, NOUT + 1],
        "g1": [128, NOUT + 1],
        "ident": [128, 128],
        "b1p": [128, 1],
    }
    cd = {k: nc.dram_tensor(k, s, F32, kind="ExternalInput").ap() for k, s in cshapes.items()}

    x_r = x_d.rearrange("(t s p) d -> t s p d", s=4, p=128)
    out_r = out_d.rearrange("(t s p) i j -> t s p (i j)", s=4, p=128)

    def mm(out, lhsT, rhs, **kw):
        nc.tensor.matmul(out, lhsT.bitcast(mm_dt), rhs.bitcast(mm_dt), **kw)

    with tile.TileContext(nc) as tc:
        with ExitStack() as ctx:
            consts = ctx.enter_context(tc.tile_pool(name="consts", bufs=1))
            io_x = ctx.enter_context(tc.tile_pool(name="io_x", bufs=3))
            io_o = ctx.enter_context(tc.tile_pool(name="io_o", bufs=3))
            sb_xt = ctx.enter_context(tc.tile_pool(name="sb_xt", bufs=2))
            sb_h = ctx.enter_context(tc.tile_pool(name="sb_h", bufs=2))
            sb_u = ctx.enter_context(tc.tile_pool(name="sb_u", bufs=2))
            sb_t = ctx.enter_context(tc.tile_pool(name="sb_t", bufs=2))
            sb_rt = ctx.enter_context(tc.tile_pool(name="sb_rt", bufs=4))
            ps_xt = ctx.enter_context(tc.tile_pool(name="ps_xt", bufs=2, space="PSUM"))
            ps_h = ctx.enter_context(tc.tile_pool(name="ps_h", bufs=1, space="PSUM"))
            ps_w = ctx.enter_context(tc.tile_pool(name="ps_w", bufs=2, space="PSUM"))
            ps_t = ctx.enter_context(tc.tile_pool(name="ps_t", bufs=2, space="PSUM"))
            ps_o = ctx.enter_context(tc.tile_pool(name="ps_o", bufs=1, space="PSUM"))

            c_sb = {}
            for k, s in cshapes.items():
                c_sb[k] = consts.tile(s, F32, tag=k)
                nc.sync.dma_start(out=c_sb[k], in_=cd[k])

            for t in range(NTILES):
                # -- load x tile [128, 4, 256]
                xs = io_x.tile([128, 4, IN_DIM], F32, tag="xs")
                nc.sync.dma_start(out=xs, in_=x_r[t].rearrange("s p d -> p s d"))

                # -- transpose to x^T chunks [128f, 512b] (PE, fp32)
                xt_sb = sb_xt.tile([128, 2, NB], F32, tag="xt")
                for f in range(2):
                    xt_ps = ps_xt.tile([128, NB], F32, tag="xt_ps")
                    for s in range(4):
                        nc.tensor.transpose(
                            xt_ps[:, s * 128 : (s + 1) * 128],
                            xs[:, s, f * 128 : (f + 1) * 128],
                            c_sb["ident"],
                        )
                    eng = nc.scalar if f == 0 else nc.vector
                    eng.tensor_copy(xt_sb[:, f, :], xt_ps)

                # -- h^T = relu(W1^T x^T + b1)  [128, 512]
                h_ps = ps_h.tile([128, NB], F32, tag="h")
                for c in range(2):
                    mm(h_ps, c_sb["w1c"][:, c, :], xt_sb[:, c, :],
                       start=(c == 0), stop=(c == 1))
                h_sb = sb_h.tile([128, NB], F32, tag="h")
                nc.scalar.activation(
                    h_sb, h_ps, mybir.ActivationFunctionType.Relu,
                    bias=c_sb["b1p"],
                )

                # -- channels w = A^T h (+d), squared -> u  [128, 7, 512]
                u_sb = sb_u.tile([128, NCHUNK, NB], F32, tag="u")
                for c in range(NCHUNK):
                    w_ps = ps_w.tile([128, NB], F32, tag="w")
                    mm(w_ps, c_sb["a_mat"][:, c * 128 : (c + 1) * 128], h_sb)
                    if bias_d_nonzero:
                        nc.scalar.activation(
                            u_sb[:, c, :], w_ps,
                            mybir.ActivationFunctionType.Square,
                            bias=c_sb["dbias"][:, c : c + 1],
                        )
                    elif c % 2 == 0:
                        nc.scalar.activation(
                            u_sb[:, c, :], w_ps,
                            mybir.ActivationFunctionType.Square,
                        )
                    else:
                        nc.vector.tensor_mul(u_sb[:, c, :], w_ps, w_ps)

                # -- rho rows (feature-major): strict-lower + diag/trace
                tl_ps = ps_t.tile([128, NB], F32, tag="tl")
                for c in range(NCHUNK):
                    mm(tl_ps[:NLOW, :], c_sb["clow"][:, c, :], u_sb[:, c, :],
                       start=(c == 0), stop=(c == NCHUNK - 1))
                td_ps = ps_t.tile([128, NB], F32, tag="td")
                for c in range(2):
                    mm(td_ps[:17, :], c_sb["cdiag"][:, c, :], u_sb[:, c, :],
                       start=(c == 0), stop=(c == 1))
                tl_sb = sb_t.tile([128, NB], F32, tag="tl")
                td_sb = sb_t.tile([128, NB], F32, tag="td")
                nc.scalar.tensor_copy(tl_sb[:NLOW, :], tl_ps[:NLOW, :])
                nc.vector.tensor_copy(td_sb[:17, :], td_ps[:17, :])

                # -- gather-transpose to batch-major + divide by trace
                ob = io_o.tile([128, 4, NOUT], F32, tag="ob")
                for s in range(4):
                    o_ps = ps_o.tile([128, NOUT + 1], F32, tag="o")
                    mm(o_ps, tl_sb[:NLOW, s * 128 : (s + 1) * 128],
                       c_sb["g0"][:NLOW, :], start=True, stop=False)
                    mm(o_ps, td_sb[:17, s * 128 : (s + 1) * 128],
                       c_sb["g1"][:17, :], start=False, stop=True)
                    rt = sb_rt.tile([128, 1], F32, tag="rt")
                    nc.vector.reciprocal(rt, o_ps[:, NOUT : NOUT + 1])
                    nc.vector.tensor_scalar_mul(ob[:, s, :], o_ps[:, :NOUT], rt)

                nc.sync.dma_start(out=out_r[t].rearrange("s p c -> p s c"), in_=ob)

    nc.compile()
    return nc


_PROG_CACHE = {}


def _get_program(bias_d_nonzero):
    key = bias_d_nonzero
    if key not in _PROG_CACHE:
        _PROG_CACHE[key] = build_program(bias_d_nonzero)
    return _PROG_CACHE[key]


def run(inputs, trace=False):
    x = np.ascontiguousarray(np.asarray(inputs["x"], np.float32))
    consts, dnz = build_constants(
        inputs["W1"], inputs["b1"], inputs["W2"], inputs["b2"]
    )
    nc = _get_program(dnz)
    in_maps = []
    for i in range(NCORES):
        m = {"x": np.ascontiguousarray(x[i * B_CORE : (i + 1) * B_CORE])}
        for k, v in consts.items():
            m[k] = v
        in_maps.append(m)
    res = run_bass_kernel_spmd(nc, in_maps, core_ids=list(range(NCORES)), trace=trace)
    out = np.concatenate([r["out"] for r in res.results], axis=0)
    return out, res


def kernel(**inputs):
    out, _ = run(inputs, trace=False)
    return out


# revision 13
# speedup vs baseline: 1.3799x; 1.3799x over previous
"""Trainium2 Bass kernel for DensityMatrixMLP.

Computes, for each batch row b of x [B=131072, 256]:
    h   = relu(x @ W1 + b1)            # [128]
    v   = h @ W2 + b2                  # [136] = tril entries of L [16,16]
    rho = L @ L^T                      # [16,16]
    out = rho / trace(rho)

Strategy (pure data parallel over 8 NeuronCores, 16384 rows/core):
The per-batch Gram L@L^T is a fixed quadratic map of v. Every needed
product v_a*v_b is obtained from squares of *linear* channels
(polarization):  v_a*v_b = ((v_a+v_b)^2 - v_a^2 - v_b^2)/2.
So the whole kernel becomes constant-weight matmuls + one elementwise
square, which stream batch columns through the PE at full rate:

  x^T   (PE transpose)                               [256, NB]
  h^T  = relu(W1^T x^T + b1)                         [128, NB]
  w    = A^T h + d   (816 linear channels:           [816, NB]
         136 v-channels + 680 pairwise-sum channels; A = W2-derived)
  u    = (w + d)^2   (ACT/DVE elementwise square)    [816, NB]
  Tlow = C_low^T u   (120 strict-lower rho entries)  [120, NB]
  Tdia = C_diag^T u  (16 diag entries + trace)       [17, NB]
  out  = gather-transpose: per 128-batch chunk, matmul with 0/1
         gather matrices G0/G1 maps feature-major rows to batch-major
         [128b, 257] (256 rho entries + trace), then divide by trace.

All big matmuls run in fp16 (1 cyc/row on PE like bf16 -- fp32
streams at 2 cyc/row -- but with a 10-bit mantissa; all values here
are within fp16 range). PSUM accumulation is fp32. x is transposed via
the DMA xbar (2-byte) instead of the PE.
"""

import sys

if "/opt/trn_rl_repo" not in sys.path:
    sys.path.insert(0, "/opt/trn_rl_repo")

from contextlib import ExitStack

import numpy as np

import concourse.bass as bass
import concourse.tile as tile
from concourse import bacc, mybir
from concourse.bass_utils import run_bass_kernel_spmd

# Problem shapes (hardcoded per spec).
BATCH = 131072
IN_DIM = 256
HID = 128
DIM = 16
TRIL = DIM * (DIM + 1) // 2  # 136
NCORES = 8
B_CORE = BATCH // NCORES  # 16384
NB = 512  # batch macro-tile (columns streamed per matmul)
NTILES = B_CORE // NB  # 32

# Channel space: 136 v-channels + 680 cross-sum channels.
NCROSS = sum(j + 1 for i in range(DIM) for j in range(i))  # 680
NCH = TRIL + NCROSS  # 816
NCH_PAD = 896  # 7 chunks of 128
NCHUNK = 7
NLOW = DIM * (DIM - 1) // 2  # 120
NOUT = DIM * DIM  # 256

F32 = mybir.dt.float32
F32R = mybir.dt.float32r
BF16 = mybir.dt.bfloat16
F16 = mybir.dt.float16


def _tidx(i, k):
    return i * (i + 1) // 2 + k


def _lowidx(i, j):
    return i * (i - 1) // 2 + j


def _cross_pairs():
    """Enumerate cross channels: (i, j, k) with i>j, k<=j."""
    out = []
    for i in range(DIM):
        for j in range(i):
            for k in range(j + 1):
                out.append((i, j, k))
    return out


def build_constants(W1, b1, W2, b2):
    """Host-side constant tensors, all laid out [128 partitions, free]."""
    W1 = np.asarray(W1, np.float32)
    b1 = np.asarray(b1, np.float32)
    W2 = np.asarray(W2, np.float32)
    b2 = np.asarray(b2, np.float32)
    pairs = _cross_pairs()

    # A [HID, NCH_PAD]: channel weights; d [NCH_PAD]: channel bias.
    A = np.zeros((HID, NCH_PAD), np.float32)
    d = np.zeros(NCH_PAD, np.float32)
    A[:, :TRIL] = W2
    d[:TRIL] = b2
    for m, (i, j, k) in enumerate(pairs):
        a, b = _tidx(i, k), _tidx(j, k)
        A[:, TRIL + m] = W2[:, a] + W2[:, b]
        d[TRIL + m] = b2[a] + b2[b]

    # C_low [NCH_PAD, 128]: 120 strict-lower rows + 8 zero pad rows
    # (padded so the PSUM output covers all 128 partitions -> no junk).
    C_low = np.zeros((NCH_PAD, 128), np.float32)
    for m, (i, j, k) in enumerate(pairs):
        r = _lowidx(i, j)
        a, b = _tidx(i, k), _tidx(j, k)
        C_low[TRIL + m, r] += 0.5
        C_low[a, r] -= 0.5
        C_low[b, r] -= 0.5

    # C_diag [256 (2 chunks), 128]: cols 0..15 = rho_ii, col 16 = trace,
    # cols 17..127 zero pad.
    C_diag = np.zeros((2 * 128, 128), np.float32)
    for i in range(DIM):
        for k in range(i + 1):
            C_diag[_tidx(i, k), i] = 1.0
    C_diag[:TRIL, 16] = 1.0

    # Gather matrices for the final feature-major -> batch-major matmul.
    # 258 output columns: 256 rho entries, col 256 = trace, col 257 = pad
    # (fp32r matmuls require an even moving free size).
    G0 = np.zeros((128, NOUT + 2), np.float32)  # rows 0..119 used
    G1 = np.zeros((128, NOUT + 2), np.float32)  # rows 0..16 used
    for i in range(DIM):
        for j in range(DIM):
            p = i * DIM + j
            if i > j:
                G0[_lowidx(i, j), p] = 1.0
            elif i < j:
                G0[_lowidx(j, i), p] = 1.0
            else:
                G1[i, p] = 1.0
    G1[16, NOUT] = 1.0  # trace passthrough

    # SBUF-friendly packing: [128, ...free], chunk index in free dim.
    w1c = np.zeros((128, 2, HID), np.float32)
    for c in range(2):
        w1c[:, c, :] = W1[c * 128 : (c + 1) * 128, :]
    A_p = A  # already [128, 896]
    dbias = np.zeros((128, NCHUNK), np.float32)
    for c in range(NCHUNK):
        dbias[:, c] = d[c * 128 : (c + 1) * 128]
    clow = np.zeros((128, NCHUNK, 128), np.float32)
    for c in range(NCHUNK):
        clow[:, c, :] = C_low[c * 128 : (c + 1) * 128, :]
    cdiag = np.zeros((128, 2, 128), np.float32)
    for c in range(2):
        cdiag[:, c, :] = C_diag[c * 128 : (c + 1) * 128, :]
    b1p = b1.reshape(128, 1).astype(np.float32)

    import ml_dtypes

    bf = lambda a: np.ascontiguousarray(a.astype(np.float16))
    return {
        "ident": bf(np.eye(128, dtype=np.float32)),
        "w1c": bf(w1c),
        "a_mat": bf(A_p),
        "dbias": dbias,
        "clow": bf(clow),
        "cdiag": bf(cdiag),
        "g0": bf(G0),
        "g1": bf(G1),
        "b1p": b1p,
    }, bool(np.any(d != 0.0))


def emulate(x, consts):
    """Numpy emulation of the kernel math (for constant validation)."""
    w1c = consts["w1c"].astype(np.float32)
    W1 = np.concatenate([w1c[:, 0, :], w1c[:, 1, :]], axis=0)
    h = np.maximum(x @ W1 + consts["b1p"].ravel(), 0.0)
    A = consts["a_mat"].astype(np.float32)
    d = consts["dbias"].T.ravel()  # [7*128] chunk-major == channel index
    w = h @ A + d
    u = w * w
    C_low = consts["clow"].transpose(1, 0, 2).reshape(NCH_PAD, 128).astype(np.float32)
    C_diag = consts["cdiag"].transpose(1, 0, 2).reshape(256, 128).astype(np.float32)
    w1cf = consts["w1c"].astype(np.float32)
    tlow = u @ C_low
    tdia = u[:, :256] @ C_diag
    g0 = consts["g0"].astype(np.float32)
    g1 = consts["g1"].astype(np.float32)
    o = tlow @ g0 + tdia @ g1
    return (o[:, :NOUT] / o[:, NOUT : NOUT + 1]).reshape(-1, DIM, DIM)


def build_program(bias_d_nonzero, mm_dt=F16):
    """Build the Bass/Tile program (value-independent)."""
    nc = bacc.Bacc("TRN2", target_bir_lowering=False, debug=False)

    x_d = nc.dram_tensor("x", [B_CORE, IN_DIM], F32, kind="ExternalInput").ap()
    out_d = nc.dram_tensor("out", [B_CORE, DIM, DIM], F32, kind="ExternalOutput").ap()
    cshapes = {
        "w1c": [128, 2, HID],
        "a_mat": [128, NCH_PAD],
        "dbias": [128, NCHUNK],
        "clow": [128, NCHUNK, 128],
        "cdiag": [128, 2, 128],
        "g0": [128, NOUT + 2],
        "g1": [128, NOUT + 2],
        "ident": [128, 128],
        "b1p": [128, 1],
    }
    MM_CONSTS = {"w1c", "a_mat", "clow", "cdiag", "g0", "g1", "ident"}
    cd = {
        k: nc.dram_tensor(k, s, mm_dt if k in MM_CONSTS else F32, kind="ExternalInput").ap()
        for k, s in cshapes.items()
    }

    x_r = x_d.rearrange("(t s p) d -> t s p d", s=4, p=128)
    out_r = out_d.rearrange("(t s p) i j -> t s p (i j)", s=4, p=128)

    mm = nc.tensor.matmul

    with tile.TileContext(nc) as tc:
        with ExitStack() as ctx:
            consts = ctx.enter_context(tc.tile_pool(name="consts", bufs=1))
            io_x = ctx.enter_context(tc.tile_pool(name="io_x", bufs=3))
            io_o = ctx.enter_context(tc.tile_pool(name="io_o", bufs=3))
            sb_xb = ctx.enter_context(tc.tile_pool(name="sb_xb", bufs=2))
            sb_xt = ctx.enter_context(tc.tile_pool(name="sb_xt", bufs=2))
            sb_h = ctx.enter_context(tc.tile_pool(name="sb_h", bufs=2))
            sb_u = ctx.enter_context(tc.tile_pool(name="sb_u", bufs=2))
            sb_t = ctx.enter_context(tc.tile_pool(name="sb_t", bufs=2))
            sb_rt = ctx.enter_context(tc.tile_pool(name="sb_rt", bufs=4))
            ps_xt = ctx.enter_context(tc.tile_pool(name="ps_xt", bufs=1, space="PSUM"))
            ps_h = ctx.enter_context(tc.tile_pool(name="ps_h", bufs=1, space="PSUM"))
            ps_w = ctx.enter_context(tc.tile_pool(name="ps_w", bufs=2, space="PSUM"))
            ps_t = ctx.enter_context(tc.tile_pool(name="ps_t", bufs=1, space="PSUM"))
            ps_o = ctx.enter_context(tc.tile_pool(name="ps_o", bufs=2, space="PSUM"))

            c_sb = {}
            for k, sh in cshapes.items():
                c_sb[k] = consts.tile(
                    sh, mm_dt if k in MM_CONSTS else F32, tag=k, name=f"c_{k}"
                )
                nc.sync.dma_start(out=c_sb[k], in_=cd[k])

            for t in range(NTILES):
                # -- load x tile [128, 4, 256], casting fp32->fp16 in the DMA
                xb = sb_xb.tile([128, 4, IN_DIM], mm_dt, tag="xb")
                nc.gpsimd.dma_start(out=xb, in_=x_r[t].rearrange("s p d -> p s d"))

                # -- x^T via PE transpose (fp16, 1 cyc/row), one PSUM bank
                xt_ps = ps_xt.tile([128, 2, NB], mm_dt, tag="xt_ps")
                for f in range(2):
                    for sub in range(4):
                        nc.tensor.transpose(
                            xt_ps[:, f, sub * 128 : (sub + 1) * 128],
                            xb[:, sub, f * 128 : (f + 1) * 128],
                            c_sb["ident"],
                        )
                xt_sb = sb_xt.tile([128, 2, NB], mm_dt, tag="xt")
                nc.vector.tensor_copy(xt_sb, xt_ps)

                # -- h^T = relu(W1^T x^T + b1)  [128, 512]
                h_ps = ps_h.tile([128, NB], F32, tag="h")
                for c in range(2):
                    mm(h_ps, c_sb["w1c"][:, c, :], xt_sb[:, c, :],
                       start=(c == 0), stop=(c == 1))
                h_sb = sb_h.tile([128, NB], mm_dt, tag="h")
                nc.scalar.activation(
                    h_sb, h_ps, mybir.ActivationFunctionType.Relu,
                    bias=c_sb["b1p"],
                )

                # -- channels w = A^T h (+d), squared -> u  [128, 7, 512] bf16
                u_sb = sb_u.tile([128, NCHUNK, NB], mm_dt, tag="u")
                for c in range(NCHUNK):
                    w_ps = ps_w.tile([128, NB], F32, tag="w")
                    mm(w_ps, c_sb["a_mat"][:, c * 128 : (c + 1) * 128], h_sb)
                    nc.scalar.activation(
                        u_sb[:, c, :], w_ps,
                        mybir.ActivationFunctionType.Square,
                        bias=c_sb["dbias"][:, c : c + 1] if bias_d_nonzero else 0.0,
                    )

                # -- rho rows (feature-major): strict-lower + diag/trace
                tl_ps = ps_t.tile([128, NB], F32, tag="tl")
                for c in range(NCHUNK):
                    mm(tl_ps, c_sb["clow"][:, c, :], u_sb[:, c, :],
                       start=(c == 0), stop=(c == NCHUNK - 1))
                td_ps = ps_t.tile([128, NB], F32, tag="td")
                for c in range(2):
                    mm(td_ps, c_sb["cdiag"][:, c, :], u_sb[:, c, :],
                       start=(c == 0), stop=(c == 1))
                tl_sb = sb_t.tile([128, NB], mm_dt, tag="tl")
                td_sb = sb_t.tile([128, NB], mm_dt, tag="td")
                nc.vector.tensor_copy(tl_sb, tl_ps)
                nc.vector.tensor_copy(td_sb, td_ps)

                # -- gather-transpose to batch-major + divide by trace
                ob = io_o.tile([128, 4, NOUT], F32, tag="ob")
                for sub in range(4):
                    o_ps = ps_o.tile([128, NOUT + 2], F32, tag="o")
                    mm(o_ps, tl_sb[:, sub * 128 : (sub + 1) * 128],
                       c_sb["g0"], start=True, stop=False)
                    mm(o_ps, td_sb[:, sub * 128 : (sub + 1) * 128],
                       c_sb["g1"], start=False, stop=True)
                    rt = sb_rt.tile([128, 1], F32, tag="rt")
                    nc.vector.reciprocal(rt, o_ps[:, NOUT : NOUT + 1])
                    nc.vector.tensor_scalar_mul(ob[:, sub, :], o_ps[:, :NOUT], rt)

                nc.sync.dma_start(out=out_r[t].rearrange("s p c -> p s c"), in_=ob)

    nc.compile()
    return nc


_PROG_CACHE = {}


def _get_program(bias_d_nonzero):
    key = bias_d_nonzero
    if key not in _PROG_CACHE:
        _PROG_CACHE[key] = build_program(bias_d_nonzero)
    return _PROG_CACHE[key]


def run(inputs, trace=False):
    x = np.ascontiguousarray(np.asarray(inputs["x"], np.float32))
    consts, dnz = build_constants(
        inputs["W1"], inputs["b1"], inputs["W2"], inputs["b2"]
    )
    nc = _get_program(dnz)
    in_maps = []
    for i in range(NCORES):
        m = {"x": np.ascontiguousarray(x[i * B_CORE : (i + 1) * B_CORE])}
        for k, v in consts.items():
            m[k] = v
        in_maps.append(m)
    res = run_bass_kernel_spmd(nc, in_maps, core_ids=list(range(NCORES)), trace=trace)
    out = np.concatenate([r["out"] for r in res.results], axis=0)
    return out, res


def kernel(**inputs):
    out, _ = run(inputs, trace=False)
    return out


# revision 14
# speedup vs baseline: 1.4573x; 1.0561x over previous
"""Trainium2 Bass kernel for DensityMatrixMLP.

Computes, for each batch row b of x [B=131072, 256]:
    h   = relu(x @ W1 + b1)            # [128]
    v   = h @ W2 + b2                  # [136] = tril entries of L [16,16]
    rho = L @ L^T                      # [16,16]
    out = rho / trace(rho)

Strategy (pure data parallel over 8 NeuronCores, 16384 rows/core):
The per-batch Gram L@L^T is a fixed quadratic map of v. Every needed
product v_a*v_b is obtained from squares of *linear* channels
(polarization):  v_a*v_b = ((v_a+v_b)^2 - v_a^2 - v_b^2)/2.
So the whole kernel becomes constant-weight matmuls + one elementwise
square, which stream batch columns through the PE at full rate:

  x^T   (PE transpose)                               [256, NB]
  h^T  = relu(W1^T x^T + b1)                         [128, NB]
  w    = A^T h + d   (816 linear channels:           [816, NB]
         136 v-channels + 680 pairwise-sum channels; A = W2-derived)
  u    = (w + d)^2   (ACT/DVE elementwise square)    [816, NB]
  Tlow = C_low^T u   (120 strict-lower rho entries)  [120, NB]
  Tdia = C_diag^T u  (16 diag entries + trace)       [17, NB]
  out  = gather-transpose: per 128-batch chunk, matmul with 0/1
         gather matrices G0/G1 maps feature-major rows to batch-major
         [128b, 257] (256 rho entries + trace), then divide by trace.

All big matmuls run in fp16 (1 cyc/row on PE like bf16 -- fp32
streams at 2 cyc/row -- but with a 10-bit mantissa; all values here
are within fp16 range). PSUM accumulation is fp32. x is transposed via
the DMA xbar (2-byte) instead of the PE.
"""

import sys

if "/opt/trn_rl_repo" not in sys.path:
    sys.path.insert(0, "/opt/trn_rl_repo")

from contextlib import ExitStack

import numpy as np

import concourse.bass as bass
import concourse.tile as tile
from concourse import bacc, mybir
from concourse.bass_utils import run_bass_kernel_spmd

# Problem shapes (hardcoded per spec).
BATCH = 131072
IN_DIM = 256
HID = 128
DIM = 16
TRIL = DIM * (DIM + 1) // 2  # 136
NCORES = 8
B_CORE = BATCH // NCORES  # 16384
NB = 512  # batch macro-tile (columns streamed per matmul)
NTILES = B_CORE // NB  # 32

# Channel space: 136 v-channels + 680 cross-sum channels.
NCROSS = sum(j + 1 for i in range(DIM) for j in range(i))  # 680
NCH = TRIL + NCROSS  # 816
NCH_PAD = 896  # 7 chunks of 128
NCHUNK = 7
NLOW = DIM * (DIM - 1) // 2  # 120
NOUT = DIM * DIM  # 256

F32 = mybir.dt.float32
F32R = mybir.dt.float32r
BF16 = mybir.dt.bfloat16
F16 = mybir.dt.float16


def _tidx(i, k):
    return i * (i + 1) // 2 + k


def _lowidx(i, j):
    return i * (i - 1) // 2 + j


def _cross_pairs():
    """Enumerate cross channels: (i, j, k) with i>j, k<=j."""
    out = []
    for i in range(DIM):
        for j in range(i):
            for k in range(j + 1):
                out.append((i, j, k))
    return out


def build_constants(W1, b1, W2, b2):
    """Host-side constant tensors, all laid out [128 partitions, free]."""
    W1 = np.asarray(W1, np.float32)
    b1 = np.asarray(b1, np.float32)
    W2 = np.asarray(W2, np.float32)
    b2 = np.asarray(b2, np.float32)
    pairs = _cross_pairs()

    # A [HID, NCH_PAD]: channel weights; d [NCH_PAD]: channel bias.
    A = np.zeros((HID, NCH_PAD), np.float32)
    d = np.zeros(NCH_PAD, np.float32)
    A[:, :TRIL] = W2
    d[:TRIL] = b2
    for m, (i, j, k) in enumerate(pairs):
        a, b = _tidx(i, k), _tidx(j, k)
        A[:, TRIL + m] = W2[:, a] + W2[:, b]
        d[TRIL + m] = b2[a] + b2[b]

    # C [NCH_PAD, 128]: cols 0..119 = strict-lower rho entries,
    # cols 120..127 = diag rho_ii for i=8..15. (Diag i<=7 is produced in
    # the gather stage straight from u chunk 0 -- all its channels are
    # tril indices < 36.)
    C_low = np.zeros((NCH_PAD, 128), np.float32)
    for m, (i, j, k) in enumerate(pairs):
        r = _lowidx(i, j)
        a, b = _tidx(i, k), _tidx(j, k)
        C_low[TRIL + m, r] += 0.5
        C_low[a, r] -= 0.5
        C_low[b, r] -= 0.5
    for i in range(8, DIM):
        for k in range(i + 1):
            C_low[_tidx(i, k), 120 + (i - 8)] = 1.0

    # Gather matrices for the final feature-major -> batch-major matmul.
    # 258 output columns: 256 rho entries, col 256 = trace, col 257 = pad
    # (4-byte/fp16 matmuls want an even moving free size).
    # G0 rows = the 128 T rows (120 low + 8 high-diag).
    # G1 rows = u chunk 0 (diag i<=7 contributions).
    G0 = np.zeros((128, NOUT + 2), np.float32)
    G1 = np.zeros((128, NOUT + 2), np.float32)
    for i in range(DIM):
        for j in range(DIM):
            p = i * DIM + j
            if i > j:
                G0[_lowidx(i, j), p] = 1.0
            elif i < j:
                G0[_lowidx(j, i), p] = 1.0
            elif i >= 8:
                G0[120 + (i - 8), p] = 1.0
    for i in range(8, DIM):
        G0[120 + (i - 8), NOUT] = 1.0  # trace: high diag part
    for i in range(8):
        for k in range(i + 1):
            a = _tidx(i, k)
            G1[a, i * DIM + i] = 1.0
            G1[a, NOUT] = 1.0  # trace: low diag part

    # SBUF-friendly packing: [128, ...free], chunk index in free dim.
    w1c = np.zeros((128, 2, HID), np.float32)
    for c in range(2):
        w1c[:, c, :] = W1[c * 128 : (c + 1) * 128, :]
    A_p = A  # already [128, 896]
    dbias = np.zeros((128, NCHUNK), np.float32)
    for c in range(NCHUNK):
        dbias[:, c] = d[c * 128 : (c + 1) * 128]
    clow = np.zeros((128, NCHUNK, 128), np.float32)
    for c in range(NCHUNK):
        clow[:, c, :] = C_low[c * 128 : (c + 1) * 128, :]
    b1p = b1.reshape(128, 1).astype(np.float32)

    import ml_dtypes

    bf = lambda a: np.ascontiguousarray(a.astype(np.float16))
    return {
        "ident": bf(np.eye(128, dtype=np.float32)),
        "w1c": bf(w1c),
        "a_mat": bf(A_p),
        "dbias": dbias,
        "clow": bf(clow),
        "g0": bf(G0),
        "g1": bf(G1),
        "b1p": b1p,
    }, bool(np.any(d != 0.0))


def emulate(x, consts):
    """Numpy emulation of the kernel math (for constant validation)."""
    w1c = consts["w1c"].astype(np.float32)
    W1 = np.concatenate([w1c[:, 0, :], w1c[:, 1, :]], axis=0)
    h = np.maximum(x @ W1 + consts["b1p"].ravel(), 0.0)
    A = consts["a_mat"].astype(np.float32)
    d = consts["dbias"].T.ravel()  # [7*128] chunk-major == channel index
    w = h @ A + d
    u = w * w
    C_low = consts["clow"].transpose(1, 0, 2).reshape(NCH_PAD, 128).astype(np.float32)
    tlow = u @ C_low
    g0 = consts["g0"].astype(np.float32)
    g1 = consts["g1"].astype(np.float32)
    o = tlow @ g0 + u[:, :128] @ g1
    return (o[:, :NOUT] / o[:, NOUT : NOUT + 1]).reshape(-1, DIM, DIM)


def build_program(bias_d_nonzero, mm_dt=F16):
    """Build the Bass/Tile program (value-independent)."""
    nc = bacc.Bacc("TRN2", target_bir_lowering=False, debug=False)

    x_d = nc.dram_tensor("x", [B_CORE, IN_DIM], F32, kind="ExternalInput").ap()
    out_d = nc.dram_tensor("out", [B_CORE, DIM, DIM], F32, kind="ExternalOutput").ap()
    cshapes = {
        "w1c": [128, 2, HID],
        "a_mat": [128, NCH_PAD],
        "dbias": [128, NCHUNK],
        "clow": [128, NCHUNK, 128],
        "g0": [128, NOUT + 2],
        "g1": [128, NOUT + 2],
        "ident": [128, 128],
        "b1p": [128, 1],
    }
    MM_CONSTS = {"w1c", "a_mat", "clow", "g0", "g1", "ident"}
    cd = {
        k: nc.dram_tensor(k, s, mm_dt if k in MM_CONSTS else F32, kind="ExternalInput").ap()
        for k, s in cshapes.items()
    }

    x_r = x_d.rearrange("(t s p) d -> t s p d", s=4, p=128)
    out_r = out_d.rearrange("(t s p) i j -> t s p (i j)", s=4, p=128)

    mm = nc.tensor.matmul

    with tile.TileContext(nc) as tc:
        with ExitStack() as ctx:
            consts = ctx.enter_context(tc.tile_pool(name="consts", bufs=1))
            io_x = ctx.enter_context(tc.tile_pool(name="io_x", bufs=3))
            io_o = ctx.enter_context(tc.tile_pool(name="io_o", bufs=3))
            sb_xb = ctx.enter_context(tc.tile_pool(name="sb_xb", bufs=2))
            sb_xt = ctx.enter_context(tc.tile_pool(name="sb_xt", bufs=2))
            sb_h = ctx.enter_context(tc.tile_pool(name="sb_h", bufs=2))
            sb_u = ctx.enter_context(tc.tile_pool(name="sb_u", bufs=2))
            sb_t = ctx.enter_context(tc.tile_pool(name="sb_t", bufs=2))
            sb_rt = ctx.enter_context(tc.tile_pool(name="sb_rt", bufs=4))
            ps_xt = ctx.enter_context(tc.tile_pool(name="ps_xt", bufs=1, space="PSUM"))
            ps_h = ctx.enter_context(tc.tile_pool(name="ps_h", bufs=1, space="PSUM"))
            ps_w = ctx.enter_context(tc.tile_pool(name="ps_w", bufs=2, space="PSUM"))
            ps_t = ctx.enter_context(tc.tile_pool(name="ps_t", bufs=1, space="PSUM"))
            ps_o = ctx.enter_context(tc.tile_pool(name="ps_o", bufs=3, space="PSUM"))

            c_sb = {}
            for k, sh in cshapes.items():
                c_sb[k] = consts.tile(
                    sh, mm_dt if k in MM_CONSTS else F32, tag=k, name=f"c_{k}"
                )
                nc.sync.dma_start(out=c_sb[k], in_=cd[k])

            for t in range(NTILES):
                # -- load x tile [128, 4, 256], casting fp32->fp16 in the DMA
                xb = sb_xb.tile([128, 4, IN_DIM], mm_dt, tag="xb")
                nc.gpsimd.dma_start(out=xb, in_=x_r[t].rearrange("s p d -> p s d"))

                # -- x^T via PE transpose (fp16, 1 cyc/row), one PSUM bank
                xt_ps = ps_xt.tile([128, 2, NB], mm_dt, tag="xt_ps")
                for f in range(2):
                    for sub in range(4):
                        nc.tensor.transpose(
                            xt_ps[:, f, sub * 128 : (sub + 1) * 128],
                            xb[:, sub, f * 128 : (f + 1) * 128],
                            c_sb["ident"],
                        )
                xt_sb = sb_xt.tile([128, 2, NB], mm_dt, tag="xt")
                nc.vector.tensor_copy(xt_sb, xt_ps)

                # -- h^T = relu(W1^T x^T + b1)  [128, 512]
                h_ps = ps_h.tile([128, NB], F32, tag="h")
                for c in range(2):
                    mm(h_ps, c_sb["w1c"][:, c, :], xt_sb[:, c, :],
                       start=(c == 0), stop=(c == 1))
                h_sb = sb_h.tile([128, NB], mm_dt, tag="h")
                nc.scalar.activation(
                    h_sb, h_ps, mybir.ActivationFunctionType.Relu,
                    bias=c_sb["b1p"],
                )

                # -- channels w = A^T h (+d), squared -> u  [128, 7, 512] bf16
                u_sb = sb_u.tile([128, NCHUNK, NB], mm_dt, tag="u")
                for c in range(NCHUNK):
                    w_ps = ps_w.tile([128, NB], F32, tag="w")
                    mm(w_ps, c_sb["a_mat"][:, c * 128 : (c + 1) * 128], h_sb)
                    nc.scalar.activation(
                        u_sb[:, c, :], w_ps,
                        mybir.ActivationFunctionType.Square,
                        bias=c_sb["dbias"][:, c : c + 1] if bias_d_nonzero else 0.0,
                    )

                # -- rho rows (feature-major): 120 strict-lower + 8 high-diag
                tl_ps = ps_t.tile([128, NB], F32, tag="tl")
                for c in range(NCHUNK):
                    mm(tl_ps, c_sb["clow"][:, c, :], u_sb[:, c, :],
                       start=(c == 0), stop=(c == NCHUNK - 1))
                tl_sb = sb_t.tile([128, NB], mm_dt, tag="tl")
                nc.vector.tensor_copy(tl_sb, tl_ps)

                # -- gather-transpose to batch-major + divide by trace
                ob = io_o.tile([128, 4, NOUT], F32, tag="ob")
                for sub in range(4):
                    o_ps = ps_o.tile([128, NOUT + 2], F32, tag="o")
                    mm(o_ps, tl_sb[:, sub * 128 : (sub + 1) * 128],
                       c_sb["g0"], start=True, stop=False)
                    mm(o_ps, u_sb[:, 0, sub * 128 : (sub + 1) * 128],
                       c_sb["g1"], start=False, stop=True)
                    rt = sb_rt.tile([128, 1], F32, tag="rt")
                    nc.vector.reciprocal(rt, o_ps[:, NOUT : NOUT + 1])
                    nc.vector.tensor_scalar_mul(ob[:, sub, :], o_ps[:, :NOUT], rt)

                nc.sync.dma_start(out=out_r[t].rearrange("s p c -> p s c"), in_=ob)

    nc.compile()
    return nc


_PROG_CACHE = {}


def _get_program(bias_d_nonzero):
    key = bias_d_nonzero
    if key not in _PROG_CACHE:
        _PROG_CACHE[key] = build_program(bias_d_nonzero)
    return _PROG_CACHE[key]


def run(inputs, trace=False):
    x = np.ascontiguousarray(np.asarray(inputs["x"], np.float32))
    consts, dnz = build_constants(
        inputs["W1"], inputs["b1"], inputs["W2"], inputs["b2"]
    )
    nc = _get_program(dnz)
    in_maps = []
    for i in range(NCORES):
        m = {"x": np.ascontiguousarray(x[i * B_CORE : (i + 1) * B_CORE])}
        for k, v in consts.items():
            m[k] = v
        in_maps.append(m)
    res = run_bass_kernel_spmd(nc, in_maps, core_ids=list(range(NCORES)), trace=trace)
    out = np.concatenate([r["out"] for r in res.results], axis=0)
    return out, res


def kernel(**inputs):
    out, _ = run(inputs, trace=False)
    return out
